# revision 1
# baseline (speedup 1.0000x reference)
"""Multi-head attention (B=2, S=2048, D=1024, H=16, dk=dv=64) on 8 TRN2 cores.

Sharding: core c -> batch b = c % 2, head-group g = c // 2 (heads 4g..4g+3).
Each core computes its 4 heads' attention for one batch plus the partial
output projection; the host sums the 4 partials per batch and adds bo.

Host marshalling: inputs are sliced per batch, transposed to [D, S]
(the PE contracts over the partition dim, so projections need D-major
operands), and the per-head weights are packed/stacked; the reference's
softmax/dk/2 scale is folded into Wv and bv.

Per-core device pipeline (matmuls in float32r: full rate, ~13-bit mantissa):
  1. QWT/KWT [dk, S] head projections (heads pair-stacked on partitions,
     biases fused into the ACT PSUM->SBUF eviction), VW [S, dv] natural
     (bias + the softmax-denominator ones column via K=1 rank-1 matmuls).
  2. scoresT[t, s] = KWT.T @ QWT per head, two heads concurrently via
     64x128 PE row tiling; exp fused into the PSUM->SBUF eviction (ACT).
     No max-subtraction (|scores| < 40, exp stays finite in fp32).
  3. ctxT[dv+1, s] = VW1.T @ exp_scoresT accumulated over t; row dv is the
     softmax denominator. Normalize: K=1 matmul broadcasts the denominator
     row to 64 partitions, DVE reciprocal, DVE multiply (the eviction).
  4. out[s, D] partial = ctx_allT.T @ Wo_slice, ACT-evicted, DMA'd out.
"""
import os
import sys

sys.path.insert(0, "/opt/trn_rl_repo")
os.environ.setdefault("JAX_PLATFORMS", "axon,cpu")

from contextlib import ExitStack

import numpy as np

import concourse.bacc as bacc
import concourse.tile as tile
from concourse import mybir
from concourse.bass_utils import run_bass_kernel_spmd

FP32 = mybir.dt.float32
FP32R = mybir.dt.float32r

B, S, D = 2, 2048, 1024
H, DK, DV = 16, 64, 64
N_CORES = 8
HPC = H // (N_CORES // B)  # heads per core = 4
P = 128
SBLK = 512                # s-block (free dim of scores matmuls)
NBLK = S // SBLK          # 4
NTT = S // P              # 16 t-tiles
NDC = D // P              # 8 contraction chunks
NV = HPC * (DV + 1)       # 260
SCALE = 1.0 / (DK * 2.0)  # folded into Wv/bv


def _build_nc():
    nc = bacc.Bacc("TRN2", target_bir_lowering=False, debug=False,
                   num_devices=N_CORES)
    d = {}
    for name, shape in [
        ("qt", [D, S]), ("kt", [D, S]), ("vt", [D, S]),
        ("wq", [D, 2 * P]), ("wk", [D, 2 * P]), ("wv", [D, 2 * P]),
        ("bqk", [P, 6]), ("ident", [P, P]), ("onescol", [P, NTT]),
        ("wo", [HPC * DV, D]), ("ones", [1, SBLK]),
    ]:
        d[name] = nc.dram_tensor(name, shape, FP32, kind="ExternalInput").ap()
    out_d = nc.dram_tensor("out", [S, D], FP32, kind="ExternalOutput").ap()
    # [D, S] viewed as [p, dc, s] chunks for DMA
    xt_view = {
        n: d[n].rearrange("(dc p) s -> p dc s", p=P).bitcast(FP32R)
        for n in ("qt", "kt", "vt")
    }

    with tile.TileContext(nc) as tc, ExitStack() as ctx:
        const = ctx.enter_context(tc.tile_pool(name="const", bufs=1))
        wpool = ctx.enter_context(tc.tile_pool(name="wpool", bufs=1))
        xtp = ctx.enter_context(tc.tile_pool(name="xtp", bufs=2))
        projp = ctx.enter_context(tc.tile_pool(name="projp", bufs=1))
        expp = ctx.enter_context(tc.tile_pool(name="expp", bufs=1))
        ctxp = ctx.enter_context(tc.tile_pool(name="ctxp", bufs=1))
        outp = ctx.enter_context(tc.tile_pool(name="outp", bufs=2))
        smallp = ctx.enter_context(tc.tile_pool(name="smallp", bufs=2))
        psum = ctx.enter_context(tc.tile_pool(name="psum", bufs=1, space="PSUM"))

        # ---- constants / weights (wk first: K projection starts the kernel;
        # the rest trickle in behind the first K chunk loads) ----
        wk_sb = wpool.tile([P, NDC, 2 * P], FP32R)
        nc.sync.dma_start(wk_sb[:], d["wk"].rearrange("(dc p) m -> p dc m", p=P).bitcast(FP32R))
        bqk = const.tile([P, 6], FP32)
        nc.sync.dma_start(bqk[:], d["bqk"])
        ones_r = const.tile([1, SBLK], FP32R)
        nc.sync.dma_start(ones_r[:], d["ones"].bitcast(FP32R))
        ident_r = const.tile([P, P], FP32R)
        nc.sync.dma_start(ident_r[:], d["ident"].bitcast(FP32R))
        wv_sb = wpool.tile([P, NDC, 2 * P], FP32R)
        wq_sb = wpool.tile([P, NDC, 2 * P], FP32R)
        wo_sb = wpool.tile([P, 2, D], FP32R)

        def load_w(sb, name, pat="(dc p) m -> p dc m"):
            nc.sync.dma_start(sb[:], d[name].rearrange(pat, p=P).bitcast(FP32R))

        # ---- persistent activation tiles ----
        qwt = [projp.tile([P, S], FP32R, tag=f"qwt{p_}", name=f"qwt{p_}") for p_ in range(2)]
        kwt = [projp.tile([P, S], FP32R, tag=f"kwt{p_}", name=f"kwt{p_}") for p_ in range(2)]
        vwt = [projp.tile([P, S], FP32R, tag=f"vwt{p_}", name=f"vwt{p_}") for p_ in range(2)]
        vw = projp.tile([P, NTT, NV], FP32R, tag="vw")
        # softmax-denominator ones column (once, strided over the 65-wide head slots)
        for hh in range(HPC):
            nc.sync.dma_start(vw[:, :, hh * (DV + 1) + DV],
                              d["onescol"].bitcast(FP32R))
        ctx_t = [ctxp.tile([P, S], FP32R, tag=f"ctx{p_}", name=f"ctx{p_}") for p_ in range(2)]

        def load_chunk(name, ci, tag="xtk", bufs=2):
            xt = xtp.tile([P, NDC, SBLK], FP32R, tag=tag, name="xt", bufs=bufs)
            nc.sync.dma_start(xt[:], xt_view[name][:, :, ci * SBLK:(ci + 1) * SBLK])
            return xt

        def proj_qk_pair(xt, w_sb, dst, bias_col, ci, pair, tag=None):
            """Project one head-pair of a chunk into dst[pair][:, ci*SBLK:...]."""
            pq = psum.tile([P, SBLK], FP32, tag=tag or ("pj" if pair == 0 else "po"),
                           name="pq")
            for dc in range(NDC):
                nc.tensor.matmul(pq[:], lhsT=w_sb[:, dc, pair * P:(pair + 1) * P],
                                 rhs=xt[:, dc, :], start=(dc == 0), stop=(dc == NDC - 1))
            nc.scalar.activation(dst[pair][:, ci * SBLK:(ci + 1) * SBLK], pq[:],
                                 mybir.ActivationFunctionType.Identity,
                                 bias=bqk[:, bias_col + pair:bias_col + pair + 1])

        def proj_v(xt, ci):
            """VWT (pair-stacked, like Q/K), then PE-transpose into vw natural."""
            for pair in range(2):
                proj_qk_pair(xt, wv_sb, vwt, 4, ci, pair)
            for pair in range(2):
                for c in range(SBLK // P):
                    tt = ci * (SBLK // P) + c
                    tp = psum.tile([P, P], FP32R, tag="ct0" if (pair * 4 + c) % 2 == 0 else "ct1",
                                   name="tp")
                    nc.tensor.transpose(
                        tp[:], vwt[pair][:, ci * SBLK + c * P:ci * SBLK + (c + 1) * P],
                        ident_r[:])
                    nc.vector.tensor_copy(
                        vw[:, tt, :].rearrange("p (h v) -> p h v", v=DV + 1)[:, 2 * pair:2 * pair + 2, 0:DV],
                        tp[:].rearrange("p (h v) -> p h v", h=2))

        def attn_alloc(pair):
            return [psum.tile([DV + 1, SBLK], FP32, tag=f"ct{hp}", name=f"ct{hp}")
                    for hp in range(2)]

        def attn_block(pair, b, ct, fillers):
            """Per-2-t-tile pipeline: scores(k) -> exp(k) -> ctx(k), ctx chasing
            exp by one step. One 4-bank scores PSUM per step holds both heads'
            2 t-tiles, evicted by a single FD=2048 exp. `fillers` is a list of
            no-arg callables emitting extra PE work, drained one per step."""
            NK = NTT // 2
            exs = {}
            for k in range(NK + 2):
                if k < NK:
                    sc = [psum.tile([P, 2 * SBLK], FP32, tag=f"sc{hp}", name=f"sc{hp}")
                          for hp in range(2)]
                    for sub in range(2):
                        tt = k * 2 + sub
                        for hp in range(2):
                            lo, hi = hp * DK, (hp + 1) * DK
                            nc.tensor.matmul(
                                sc[hp][:, sub * SBLK:(sub + 1) * SBLK],
                                lhsT=kwt[pair][lo:hi, tt * P:(tt + 1) * P],
                                rhs=qwt[pair][lo:hi, b * SBLK:(b + 1) * SBLK],
                                start=True, stop=True)
                    ex = [expp.tile([P, 2, SBLK], FP32R, tag=f"exp{hp}", name=f"exp{hp}", bufs=3)
                          for hp in range(2)]
                    for hp in range(2):
                        nc.scalar.activation(
                            ex[hp][:], sc[hp][:].rearrange("p (u q) -> p u q", u=2),
                            mybir.ActivationFunctionType.Exp)
                    exs[k] = ex
                if fillers:
                    fillers.pop(0)()
                # ctx trails exp by 2 steps: the block's first ctx matmul waits
                # for the ct-psum slot freed by the PREVIOUS block's normalize,
                # so give that chain two steps of slack.
                kc = k - 2
                if kc >= 0:
                    ex = exs.pop(kc)
                    for sub in range(2):
                        tt = kc * 2 + sub
                        for hp in range(2):
                            hh = 2 * pair + hp
                            nc.tensor.matmul(
                                ct[hp][:], lhsT=vw[:, tt, hh * (DV + 1):(hh + 1) * (DV + 1)],
                                rhs=ex[hp][:, sub, :],
                                start=(tt == 0), stop=(tt == NTT - 1))

        def attn_normalize(pair, b, ct):
            # ctx = ct[0:64] * (1 / ct[64]) row-broadcast
            for hp in range(2):
                den = smallp.tile([1, SBLK], FP32R, tag="den")
                nc.vector.tensor_copy(den[:], ct[hp][DV:DV + 1, :])
                rb = psum.tile([DV, SBLK], FP32, tag="pj", name="rb")
                nc.tensor.matmul(rb[:], lhsT=ones_r[:, 0:DV], rhs=den[:],
                                 start=True, stop=True)
                rcp = smallp.tile([DV, SBLK], FP32, tag="rcp")
                nc.vector.reciprocal_approx_fast(rcp[:], rb[:])
                nc.vector.tensor_mul(
                    ctx_t[pair][hp * DV:(hp + 1) * DV, b * SBLK:(b + 1) * SBLK],
                    ct[hp][0:DV, :], rcp[:])

        def out_proj_nh(b, st, nh, tag="po"):
            off = b * SBLK + st * P
            po = psum.tile([P, SBLK], FP32, tag=tag, name="po")
            for jc in range(2):
                nc.tensor.matmul(po[:],
                                 lhsT=ctx_t[jc][:, off:off + P],
                                 rhs=wo_sb[:, jc, nh * SBLK:(nh + 1) * SBLK],
                                 start=(jc == 0), stop=(jc == 1))
            ob = outp.tile([P, SBLK], FP32, tag="ob")
            nc.scalar.copy(ob[:], po[:])
            nc.sync.dma_start(out_d[off:off + P, nh * SBLK:(nh + 1) * SBLK], ob[:])

        def proj_qk_piece(xt, w_sb, dst, bias_col, ci, pair, dc_range, pq_holder):
            if dc_range[0] == 0:
                pq_holder[pair] = psum.tile([P, SBLK], FP32, tag="pj", name="pq")
            pq = pq_holder[pair]
            for dc in dc_range:
                nc.tensor.matmul(pq[:], lhsT=w_sb[:, dc, pair * P:(pair + 1) * P],
                                 rhs=xt[:, dc, :], start=(dc == 0), stop=(dc == NDC - 1))
            if dc_range[-1] == NDC - 1:
                nc.scalar.activation(dst[pair][:, ci * SBLK:(ci + 1) * SBLK], pq[:],
                                     mybir.ActivationFunctionType.Identity,
                                     bias=bqk[:, bias_col + pair:bias_col + pair + 1])

        # ---- emission schedule ----
        # K and V fully first (attention needs full-T KWT/VW); Q chunk-by-chunk.
        # The next chunk's Q projection and the previous block's output
        # projection are drained into attention's per-step PE slack.
        vts = {}
        for ci in range(NBLK):
            kt = load_chunk("kt", ci)
            if ci == 0:
                load_w(wv_sb, "wv")
                vts[0] = load_chunk("vt", 0, tag="xtv", bufs=1)
            if ci == 2:
                load_w(wq_sb, "wq")
            proj_qk_pair(kt, wk_sb, kwt, 2, ci, 0)
            proj_qk_pair(kt, wk_sb, kwt, 2, ci, 1)
        for ci in range(NBLK):
            vt = vts.pop(ci) if ci in vts else load_chunk("vt", ci, tag="xtv", bufs=1)
            if ci == 0:
                load_w(wo_sb, "wo", "(jc p) n -> p jc n")
            proj_v(vt, ci)
        qt = load_chunk("qt", 0)
        proj_qk_pair(qt, wq_sb, qwt, 0, 0, 0)
        proj_qk_pair(qt, wq_sb, qwt, 0, 0, 1)
        def interleave(a, bl):
            out = []
            for i in range(max(len(a), len(bl))):
                if i < len(a):
                    out.append(a[i])
                if i < len(bl):
                    out.append(bl[i])
            return out

        prev_norm = None  # pair-1 normalize deferred into the next block
        for b in range(NBLK):
            have_next = b + 1 < NBLK
            pp = [[], []]
            if have_next:
                qt = load_chunk("qt", b + 1)
                holder = [None, None]
                for pair in range(2):
                    for dcs in ([0, 1], [2, 3], [4, 5], [6, 7]):
                        pp[pair].append(lambda xt=qt, p=pair, r=tuple(dcs), h=holder:
                                        proj_qk_piece(xt, wq_sb, qwt, 0, b + 1, p, r, h))
            op = [[], []]
            if b > 0:
                for st in range(4):
                    for nh in range(2):
                        op[st // 2].append(lambda s=st, n=nh: out_proj_nh(b - 1, s, n))
            fill0 = ([prev_norm] if prev_norm else []) + interleave(pp[0], op[0])
            ct0 = attn_alloc(0)
            attn_block(0, b, ct0, fill0)
            fill1 = [lambda bb=b, c=ct0: attn_normalize(0, bb, c)] + interleave(pp[1], op[1])
            ct1 = attn_alloc(1)
            attn_block(1, b, ct1, fill1)
            prev_norm = (lambda bb=b, c=ct1: attn_normalize(1, bb, c))
        prev_norm()
        for st in range(4):
            for nh in range(2):
                out_proj_nh(NBLK - 1, st, nh, tag="po" if (st * 2 + nh) % 2 == 0 else "pj")

    nc.compile()
    return nc


_NC_CACHE = None


def _get_nc():
    global _NC_CACHE
    if _NC_CACHE is None:
        _NC_CACHE = _build_nc()
    return _NC_CACHE


def kernel(Q, K, V, Wq, bq, Wk, bk, Wv, bv, Wo, bo, _trace=False, _trace_kwargs=None):
    nc = _get_nc()
    ones = np.ones((1, SBLK), dtype=np.float32)
    ident = np.eye(P, dtype=np.float32)
    qt_h = [np.ascontiguousarray(np.asarray(Q[b]).T) for b in range(B)]
    kt_h = [np.ascontiguousarray(np.asarray(K[b]).T) for b in range(B)]
    vt_h = [np.ascontiguousarray(np.asarray(V[b]).T) for b in range(B)]

    in_maps = []
    for c in range(N_CORES):
        b, g = c % B, c // B
        hs = list(range(g * HPC, (g + 1) * HPC))
        wq_p = np.concatenate([Wq[h] for h in hs], axis=1)
        wk_p = np.concatenate([Wk[h] for h in hs], axis=1)
        wv_p = np.concatenate([Wv[h] * SCALE for h in hs], axis=1)
        bqk_p = np.stack([
            np.concatenate([bq[hs[0]], bq[hs[1]]]),
            np.concatenate([bq[hs[2]], bq[hs[3]]]),
            np.concatenate([bk[hs[0]], bk[hs[1]]]),
            np.concatenate([bk[hs[2]], bk[hs[3]]]),
            np.concatenate([bv[hs[0]], bv[hs[1]]]) * SCALE,
            np.concatenate([bv[hs[2]], bv[hs[3]]]) * SCALE,
        ], axis=1)
        in_maps.append({
            "qt": qt_h[b], "kt": kt_h[b], "vt": vt_h[b],
            "wq": np.ascontiguousarray(wq_p),
            "wk": np.ascontiguousarray(wk_p),
            "wv": np.ascontiguousarray(wv_p),
            "bqk": np.ascontiguousarray(bqk_p.astype(np.float32)),
            "ident": ident,
            "onescol": np.ones((P, NTT), dtype=np.float32),
            "wo": np.ascontiguousarray(Wo[g * HPC * DV:(g + 1) * HPC * DV]),
            "ones": ones,
        })

    kw = {}
    if _trace:
        kw = dict(trace=True, **(_trace_kwargs or {}))
    res = run_bass_kernel_spmd(nc, in_maps, core_ids=list(range(N_CORES)), **kw)

    out = np.zeros((B, S, D), dtype=np.float32)
    for c in range(N_CORES):
        out[c % B] += res.results[c]["out"]
    out += bo[None, None, :]
    if _trace:
        return out, res
    return out



# revision 6
# speedup vs baseline: 1.2972x; 1.2972x over previous
"""Multi-head attention (B=2, S=2048, D=1024, H=16, dk=dv=64) on 8 TRN2 cores.

Sharding: core c -> batch b = c % 2, head-group g = c // 2 (heads 4g..4g+3).
Each core computes its 4 heads' attention for one batch plus the partial
output projection; the host sums the 4 partials per batch and adds bo.

v2 design (vs the transpose-heavy v1): the whole input path is fp16
(halves HBM traffic; fp16's 11-bit mantissa keeps scores to ~1e-3), the
V projection is computed directly in natural [t, dv] layout (lhsT = the
V chunk itself), and every projection bias is a rank-1 K=1 matmul into
the accumulating PSUM so all PSUM->SBUF evictions are pure DVE copies.
The ACT engine then does nothing but the softmax exp, which is its hard
floor: (1024+352)/1.2GHz per [128,1024] tile, ~147us over the 128 tiles.
The attention pipeline runs one t-tile per step (scores -> exp -> ctx
trailing 2 steps) with a 3-deep scores-PSUM rotation so the PE can run
ahead of ACT and never bubbles long enough to re-throttle the HAM clock
gate. K/V/Q chunks stream in while block 0 is already computing; Q-proj,
out-proj and normalize work drains into later blocks' per-step slack.
"""
import os
import sys

sys.path.insert(0, "/opt/trn_rl_repo")
os.environ.setdefault("JAX_PLATFORMS", "axon,cpu")

from contextlib import ExitStack

import numpy as np

import concourse.bacc as bacc
import concourse.tile as tile
from concourse import mybir
from concourse.bass_utils import run_bass_kernel_spmd

FP16 = mybir.dt.float16
FP32 = mybir.dt.float32
FP32R = mybir.dt.float32r

B, S, D = 2, 2048, 1024
H, DK, DV = 16, 64, 64
N_CORES = 8
HPC = H // (N_CORES // B)  # heads per core = 4
P = 128
SBLK = 512                # s-block (free dim of scores matmuls)
NBLK = S // SBLK          # 4
NTT = S // P              # 16 t-tiles
NDC = D // P              # 8 contraction chunks
NV = HPC * (DV + 1)       # 260 (64 V cols + 1 denominator-ones col per head)
SCALE = 1.0 / (DK * 2.0)  # folded into Wv/bv
# brow packing offsets (one [1, 1284] fp16 row of constants)
ONES_OFF, BQ_OFF, BK_OFF, BVE_OFF = 0, 512, 768, 1024
BROW_W = 1284


def _build_nc():
    nc = bacc.Bacc("TRN2", target_bir_lowering=False, debug=False,
                   num_devices=N_CORES)
    d = {}
    for name, shape, dt in [
        ("qt", [D, S], FP16), ("kt", [D, S], FP16), ("vt", [D, S], FP16),
        ("wq", [D, 2 * P], FP16), ("wk", [D, 2 * P], FP16),
        ("wv", [D, NV], FP16), ("wo", [HPC * DV, D], FP16),
        ("brow", [1, BROW_W], FP16), ("onesdv", [1, DV], FP32),
    ]:
        d[name] = nc.dram_tensor(name, shape, dt, kind="ExternalInput").ap()
    out_d = nc.dram_tensor("out", [S, D], FP16, kind="ExternalOutput").ap()
    xt_view = {
        n: d[n].rearrange("(dc p) s -> p dc s", p=P)
        for n in ("qt", "kt", "vt")
    }

    with tile.TileContext(nc) as tc, ExitStack() as ctx:
        const = ctx.enter_context(tc.tile_pool(name="const", bufs=1))
        wpool = ctx.enter_context(tc.tile_pool(name="wpool", bufs=1))
        xtp = ctx.enter_context(tc.tile_pool(name="xtp", bufs=1))
        projp = ctx.enter_context(tc.tile_pool(name="projp", bufs=1))
        expp = ctx.enter_context(tc.tile_pool(name="expp", bufs=1))
        ctxp = ctx.enter_context(tc.tile_pool(name="ctxp", bufs=1))
        outp = ctx.enter_context(tc.tile_pool(name="outp", bufs=2))
        smallp = ctx.enter_context(tc.tile_pool(name="smallp", bufs=2))
        psum = ctx.enter_context(tc.tile_pool(name="psum", bufs=1, space="PSUM"))

        # ---- ACT table warm-up: a 2-elem exp triggers ACT_TABLE_LOAD
        # while the first DMAs are still in flight.
        dummy = smallp.tile([1, 2], FP32, tag="dmy")
        dummy2 = smallp.tile([1, 2], FP32, tag="dmy2")
        nc.vector.memset(dummy[:], 0.0)
        nc.scalar.activation(dummy2[:], dummy[:],
                             mybir.ActivationFunctionType.Exp)

        # ---- constants / weights ----
        brow = const.tile([1, BROW_W], FP16)
        nc.sync.dma_start(brow[:], d["brow"])
        onesdv = const.tile([1, DV], FP32R)
        nc.sync.dma_start(onesdv[:], d["onesdv"].bitcast(FP32R))
        wk_sb = wpool.tile([P, NDC, 2 * P], FP16)
        nc.sync.dma_start(wk_sb[:], d["wk"].rearrange("(dc p) m -> p dc m", p=P))
        wq_sb = wpool.tile([P, NDC, 2 * P], FP16)
        nc.sync.dma_start(wq_sb[:], d["wq"].rearrange("(dc p) m -> p dc m", p=P))
        wv_sb = wpool.tile([P, NDC, NV], FP16)
        nc.sync.dma_start(wv_sb[:], d["wv"].rearrange("(dc p) m -> p dc m", p=P))
        wo_sb = wpool.tile([P, 2, D], FP16)

        # ---- persistent activation tiles ----
        kwt = [projp.tile([P, S], FP16, tag=f"kwt{p_}", name=f"kwt{p_}") for p_ in range(2)]
        qwt = [projp.tile([P, S], FP16, tag=f"qwt{p_}", name=f"qwt{p_}") for p_ in range(2)]
        vw = projp.tile([P, NTT, NV], FP32R, tag="vw")
        ctx_t = [ctxp.tile([P, S], FP16, tag=f"ctx{p_}", name=f"ctx{p_}") for p_ in range(2)]

        def load_chunk(name, ci):
            xt = xtp.tile([P, NDC, SBLK], FP16, tag="xt", name="xt", bufs=4)
            nc.sync.dma_start(xt[:], xt_view[name][:, :, ci * SBLK:(ci + 1) * SBLK])
            return xt

        def proj_qk_piece(xt, w_sb, dst, bias_off, ci, pair, dc_range, pq_holder):
            """Part of one head-pair x one 512-s-chunk projection; the final
            piece adds the rank-1 bias and DVE-evicts to fp16 SBUF."""
            if dc_range[0] == 0:
                pq_holder[pair] = psum.tile([P, 2, SBLK], FP32, tag="sc",
                                            name="pq", bufs=3)
            pq = pq_holder[pair]
            for dc in dc_range:
                nc.tensor.matmul(pq[:, 0, :], lhsT=w_sb[:, dc, pair * P:(pair + 1) * P],
                                 rhs=xt[:, dc, :], start=(dc == 0), stop=False)
            if dc_range[-1] == NDC - 1:
                nc.tensor.matmul(
                    pq[:, 0, :],
                    lhsT=brow[:, bias_off + pair * P:bias_off + (pair + 1) * P],
                    rhs=brow[:, ONES_OFF:ONES_OFF + SBLK],
                    start=False, stop=True)
                nc.vector.tensor_copy(dst[pair][:, ci * SBLK:(ci + 1) * SBLK],
                                      pq[:, 0, :])

        def proj_qk(xt, w_sb, dst, bias_off, ci, pair):
            h = [None, None]
            proj_qk_piece(xt, w_sb, dst, bias_off, ci, pair, range(NDC), h)

        def proj_v_tt(xt, ci, c):
            """One t-tile of the natural-layout V projection: [128 t, 260]."""
            tt = ci * (SBLK // P) + c
            pv = psum.tile([P, 2, SBLK], FP32, tag="sc", name="pv", bufs=3)
            for dc in range(NDC):
                nc.tensor.matmul(pv[:, 0, 0:NV], lhsT=xt[:, dc, c * P:(c + 1) * P],
                                 rhs=wv_sb[:, dc, :], start=(dc == 0), stop=False)
            nc.tensor.matmul(pv[:, 0, 0:NV], lhsT=brow[:, ONES_OFF:ONES_OFF + P],
                             rhs=brow[:, BVE_OFF:BVE_OFF + NV],
                             start=False, stop=True)
            nc.vector.tensor_copy(vw[:, tt, :], pv[:, 0, 0:NV])

        def attn_block(pair, b, fillers):
            """Per-t-tile pipeline: scores(k) -> exp(k) -> ctx(k-2).
            One 2-bank scores PSUM per step (hp0 | hp1), 3-deep rotation;
            exp is a single FD=1024 ACT instruction. `fillers` is a list of
            (slot, fn); fn is emitted when the step index reaches slot."""
            ct = [psum.tile([DV + 1, SBLK], FP32, tag=f"ct{hp}", name=f"ct{hp}")
                  for hp in range(2)]
            exs = {}
            for k in range(NTT + 2):
                if k < NTT:
                    sc = psum.tile([P, 2, SBLK], FP32, tag="sc", name="sc", bufs=3)
                    for hp in range(2):
                        lo, hi = hp * DK, (hp + 1) * DK
                        nc.tensor.matmul(
                            sc[:, hp, :],
                            lhsT=kwt[pair][lo:hi, k * P:(k + 1) * P],
                            rhs=qwt[pair][lo:hi, b * SBLK:(b + 1) * SBLK],
                            start=True, stop=True)
                    ex = expp.tile([P, 2, SBLK], FP32R, tag="ex", name="ex", bufs=3)
                    nc.scalar.activation(ex[:], sc[:],
                                         mybir.ActivationFunctionType.Exp)
                    exs[k] = ex
                while fillers and fillers[0][0] <= k:
                    fillers.pop(0)[1]()
                kc = k - 2
                if kc >= 0:
                    ex = exs.pop(kc)
                    for hp in range(2):
                        hh = 2 * pair + hp
                        nc.tensor.matmul(
                            ct[hp][:], lhsT=vw[:, kc, hh * (DV + 1):(hh + 1) * (DV + 1)],
                            rhs=ex[:, hp, :],
                            start=(kc == 0), stop=(kc == NTT - 1))
            return ct

        def attn_normalize(pair, b, ct, hp):
            # ctx = ct[0:64] * (1 / ct[64]) row-broadcast; fp16 out
            den = smallp.tile([1, SBLK], FP32R, tag="den")
            nc.vector.tensor_copy(den[:], ct[hp][DV:DV + 1, :])
            rb = psum.tile([P, 2, SBLK], FP32, tag="sc", name="rb", bufs=3)
            nc.tensor.matmul(rb[0:DV, 0, :], lhsT=onesdv[:],
                             rhs=den[:], start=True, stop=True)
            rcp = smallp.tile([DV, SBLK], FP32, tag="rcp")
            nc.vector.reciprocal_approx_fast(rcp[:], rb[0:DV, 0, :])
            nc.vector.tensor_mul(
                ctx_t[pair][hp * DV:(hp + 1) * DV, b * SBLK:(b + 1) * SBLK],
                ct[hp][0:DV, :], rcp[:])

        def out_proj_nh(b, st, nh):
            off = b * SBLK + st * P
            po = psum.tile([P, 2, SBLK], FP32, tag="sc", name="po", bufs=3)
            for jc in range(2):
                nc.tensor.matmul(po[:, 0, :],
                                 lhsT=ctx_t[jc][:, off:off + P],
                                 rhs=wo_sb[:, jc, nh * SBLK:(nh + 1) * SBLK],
                                 start=(jc == 0), stop=(jc == 1))
            ob = outp.tile([P, SBLK], FP16, tag="ob")
            nc.vector.tensor_copy(ob[:], po[:, 0, :])
            nc.sync.dma_start(out_d[off:off + P, nh * SBLK:(nh + 1) * SBLK], ob[:])

        # ---- emission schedule ----
        # Chunks 0-1 of K/Q/V are loaded+projected up front; block 0 pair 0
        # starts while chunks 2-3 stream in and are projected in its step
        # slack. Q chunks 1-3 / out-proj / normalize drain into later blocks.
        kt0 = load_chunk("kt", 0)
        qt0 = load_chunk("qt", 0)
        vt0 = load_chunk("vt", 0)
        proj_qk(kt0, wk_sb, kwt, BK_OFF, 0, 0)
        proj_qk(kt0, wk_sb, kwt, BK_OFF, 0, 1)
        kt1 = load_chunk("kt", 1)
        proj_qk(qt0, wq_sb, qwt, BQ_OFF, 0, 0)
        proj_qk(qt0, wq_sb, qwt, BQ_OFF, 0, 1)
        vt1 = load_chunk("vt", 1)
        for c in range(4):
            proj_v_tt(vt0, 0, c)
        kt2 = load_chunk("kt", 2)
        proj_qk(kt1, wk_sb, kwt, BK_OFF, 1, 0)
        proj_qk(kt1, wk_sb, kwt, BK_OFF, 1, 1)
        vt2 = load_chunk("vt", 2)
        for c in range(4):
            proj_v_tt(vt1, 1, c)
        kt3 = load_chunk("kt", 3)
        vt3 = load_chunk("vt", 3)
        nc.sync.dma_start(wo_sb[:], d["wo"].rearrange("(jc p) n -> p jc n", p=P))

        # block 0 pair 0: project K/V chunks 2-3 inside the step slack.
        fill = []
        for i, pr in enumerate(range(2)):
            fill.append((i * 2, lambda p=pr: proj_qk(kt2, wk_sb, kwt, BK_OFF, 2, p)))
        for c in range(4):
            fill.append((4 + c, lambda ci=2, cc=c: proj_v_tt(vt2, ci, cc)))
        for i, pr in enumerate(range(2)):
            fill.append((8 + i * 2, lambda p=pr: proj_qk(kt3, wk_sb, kwt, BK_OFF, 3, p)))
        for c in range(4):
            fill.append((11 + c, lambda ci=3, cc=c: proj_v_tt(vt3, ci, cc)))
        ct = attn_block(0, 0, fill)
        prev = (0, 0, ct)

        qts = {0: qt0}

        def q_fillers(ci, start_slot):
            """DMA was issued at block start; pieces go late in the block so
            the PE queue never parks on the DMA semaphore."""
            out = []
            slot = start_slot
            for pair in range(2):
                holder = [None, None]
                for dcs in ([0, 1, 2, 3], [4, 5, 6, 7]):
                    out.append((slot, lambda p=pair, r=tuple(dcs), h=holder, c=ci:
                                proj_qk_piece(qts[c], wq_sb, qwt, BQ_OFF, c, p, r, h)))
                    slot += 1
            return out

        # remaining 7 pair-blocks
        for b in range(NBLK):
            for pair in range(2):
                if b == 0 and pair == 0:
                    continue
                fill = []
                pp, pb, pct = prev
                fill.append((0, lambda p=pp, bb=pb, c=pct: attn_normalize(p, bb, c, 0)))
                fill.append((1, lambda p=pp, bb=pb, c=pct: attn_normalize(p, bb, c, 1)))
                if pair == 1 and b < NBLK - 1:
                    # next block's Q chunk: DMA now, project late in block
                    qts[b + 1] = load_chunk("qt", b + 1)
                    fill += q_fillers(b + 1, 10)
                if pair == 0 and b > 0:
                    for u in range(8):
                        st, nh = u // 2, u % 2
                        fill.append((2 + u, lambda bb=b - 1, s=st, n=nh:
                                     out_proj_nh(bb, s, n)))
                ct = attn_block(pair, b, fill)
                prev = (pair, b, ct)
        attn_normalize(1, NBLK - 1, ct, 0)
        attn_normalize(1, NBLK - 1, ct, 1)
        for u in range(8):
            out_proj_nh(NBLK - 1, u // 2, u % 2)

    nc.compile()
    return nc


_NC_CACHE = None


def _get_nc():
    global _NC_CACHE
    if _NC_CACHE is None:
        _NC_CACHE = _build_nc()
    return _NC_CACHE


def kernel(Q, K, V, Wq, bq, Wk, bk, Wv, bv, Wo, bo, _trace=False, _trace_kwargs=None):
    nc = _get_nc()
    qt_h = [np.ascontiguousarray(np.asarray(Q[b]).T).astype(np.float16) for b in range(B)]
    kt_h = [np.ascontiguousarray(np.asarray(K[b]).T).astype(np.float16) for b in range(B)]
    vt_h = [np.ascontiguousarray(np.asarray(V[b]).T).astype(np.float16) for b in range(B)]
    onesdv = np.ones((1, DV), dtype=np.float32)

    in_maps = []
    for c in range(N_CORES):
        b, g = c % B, c // B
        hs = list(range(g * HPC, (g + 1) * HPC))
        wq_p = np.concatenate([Wq[h] for h in hs], axis=1)
        wk_p = np.concatenate([Wk[h] for h in hs], axis=1)
        wv_e = np.zeros((D, NV), dtype=np.float32)
        bv_e = np.zeros(NV, dtype=np.float32)
        for i, h in enumerate(hs):
            wv_e[:, i * (DV + 1):i * (DV + 1) + DV] = Wv[h] * SCALE
            bv_e[i * (DV + 1):i * (DV + 1) + DV] = bv[h] * SCALE
            bv_e[i * (DV + 1) + DV] = 1.0
        brow = np.zeros((1, BROW_W), dtype=np.float32)
        brow[0, ONES_OFF:ONES_OFF + SBLK] = 1.0
        brow[0, BQ_OFF:BQ_OFF + 2 * P] = np.concatenate([bq[h] for h in hs])
        brow[0, BK_OFF:BK_OFF + 2 * P] = np.concatenate([bk[h] for h in hs])
        brow[0, BVE_OFF:BVE_OFF + NV] = bv_e
        in_maps.append({
            "qt": qt_h[b], "kt": kt_h[b], "vt": vt_h[b],
            "wq": np.ascontiguousarray(wq_p).astype(np.float16),
            "wk": np.ascontiguousarray(wk_p).astype(np.float16),
            "wv": np.ascontiguousarray(wv_e).astype(np.float16),
            "wo": np.ascontiguousarray(Wo[g * HPC * DV:(g + 1) * HPC * DV]).astype(np.float16),
            "brow": brow.astype(np.float16),
            "onesdv": onesdv,
        })

    kw = {}
    if _trace:
        kw = dict(trace=True, **(_trace_kwargs or {}))
    res = run_bass_kernel_spmd(nc, in_maps, core_ids=list(range(N_CORES)), **kw)

    out = np.zeros((B, S, D), dtype=np.float32)
    for c in range(N_CORES):
        out[c % B] += np.asarray(res.results[c]["out"], dtype=np.float32)
    out += bo[None, None, :]
    if _trace:
        return out, res
    return out


# revision 15
# speedup vs baseline: 1.5732x; 1.2128x over previous
"""Multi-head attention (B=2, S=2048, D=1024, H=16, dk=dv=64) on 8 TRN2 cores.

Sharding: core c -> batch b = c % 2, head-group g = c // 2 (heads 4g..4g+3).
Each core computes its 4 heads' attention for one batch plus the partial
output projection; the host sums the 4 partials per batch and adds bo.

v2 design (vs the transpose-heavy v1): the whole input path is fp16
(halves HBM traffic; fp16's 11-bit mantissa keeps scores to ~1e-3), the
V projection is computed directly in natural [t, dv] layout (lhsT = the
V chunk itself), and every projection bias is a rank-1 K=1 matmul into
the accumulating PSUM so all PSUM->SBUF evictions are pure DVE copies.
The ACT engine then does nothing but the softmax exp, which is its hard
floor: (1024+352)/1.2GHz per [128,1024] tile, ~147us over the 128 tiles.
The attention pipeline runs one t-tile per step (scores -> exp -> ctx
trailing 2 steps) with a 3-deep scores-PSUM rotation so the PE can run
ahead of ACT and never bubbles long enough to re-throttle the HAM clock
gate. K/V/Q chunks stream in while block 0 is already computing; Q-proj,
out-proj and normalize work drains into later blocks' per-step slack.
"""
import os
import sys

sys.path.insert(0, "/opt/trn_rl_repo")
os.environ.setdefault("JAX_PLATFORMS", "axon,cpu")

from contextlib import ExitStack

import numpy as np

import concourse.bacc as bacc
import concourse.tile as tile
from concourse import mybir
from concourse.bass_utils import run_bass_kernel_spmd

FP16 = mybir.dt.float16
BF16 = mybir.dt.bfloat16
FP32 = mybir.dt.float32
FP32R = mybir.dt.float32r

B, S, D = 2, 2048, 1024
H, DK, DV = 16, 64, 64
N_CORES = 8
HPC = H // (N_CORES // B)  # heads per core = 4
P = 128
SBLK = 512                # s-block (free dim of scores matmuls)
NBLK = S // SBLK          # 4
NTT = S // P              # 16 t-tiles
NDC = D // P              # 8 contraction chunks
NV = HPC * (DV + 1)       # 260 (64 V cols + 1 denominator-ones col per head)
SCALE = 1.0 / (DK * 2.0)  # folded into Wv/bv
# brow packing offsets (one [1, 1284] fp16 row of constants)
ONES_OFF, BQ_OFF, BK_OFF, BVE_OFF = 0, 512, 768, 1024
BROW_W = 1284


def _build_nc():
    nc = bacc.Bacc("TRN2", target_bir_lowering=False, debug=False,
                   num_devices=N_CORES)
    d = {}
    for name, shape, dt in [
        ("qt", [D, S], FP16), ("kt", [D, S], FP16), ("vt", [D, S], FP16),
        ("wq", [D, 2 * P], FP16), ("wk", [D, 2 * P], FP16),
        ("wv", [D, NV], FP16), ("wo", [HPC * DV, D], FP16),
        ("brow", [1, BROW_W], FP16), ("onesdv", [1, DV], FP32),
    ]:
        d[name] = nc.dram_tensor(name, shape, dt, kind="ExternalInput").ap()
    out_d = nc.dram_tensor("out", [S, D], FP16, kind="ExternalOutput").ap()
    xt_view = {
        n: d[n].rearrange("(dc p) s -> p dc s", p=P)
        for n in ("qt", "kt", "vt")
    }

    with tile.TileContext(nc) as tc, ExitStack() as ctx:
        const = ctx.enter_context(tc.tile_pool(name="const", bufs=1))
        wpool = ctx.enter_context(tc.tile_pool(name="wpool", bufs=1))
        xtp = ctx.enter_context(tc.tile_pool(name="xtp", bufs=1))
        projp = ctx.enter_context(tc.tile_pool(name="projp", bufs=1))
        expp = ctx.enter_context(tc.tile_pool(name="expp", bufs=1))
        ctxp = ctx.enter_context(tc.tile_pool(name="ctxp", bufs=1))
        outp = ctx.enter_context(tc.tile_pool(name="outp", bufs=2))
        smallp = ctx.enter_context(tc.tile_pool(name="smallp", bufs=2))
        psum = ctx.enter_context(tc.tile_pool(name="psum", bufs=1, space="PSUM"))

        # ---- ACT table warm-up: a 2-elem exp triggers ACT_TABLE_LOAD
        # while the first DMAs are still in flight.
        dummy = smallp.tile([1, 2], FP32, tag="dmy")
        dummy2 = smallp.tile([1, 2], FP32, tag="dmy2")
        nc.vector.memset(dummy[:], 0.0)
        nc.scalar.activation(dummy2[:], dummy[:],
                             mybir.ActivationFunctionType.Exp)

        # ---- constants / weights ----
        brow = const.tile([1, BROW_W], FP16)
        nc.sync.dma_start(brow[:], d["brow"])
        onesdv = const.tile([1, DV], FP32R)
        nc.sync.dma_start(onesdv[:], d["onesdv"].bitcast(FP32R))
        wk_sb = wpool.tile([P, NDC, 2 * P], FP16)
        nc.sync.dma_start(wk_sb[:], d["wk"].rearrange("(dc p) m -> p dc m", p=P))
        wq_sb = wpool.tile([P, NDC, 2 * P], FP16)
        nc.sync.dma_start(wq_sb[:], d["wq"].rearrange("(dc p) m -> p dc m", p=P))
        wv_sb = wpool.tile([P, NDC, NV], FP16)
        nc.sync.dma_start(wv_sb[:], d["wv"].rearrange("(dc p) m -> p dc m", p=P))
        wo_sb = wpool.tile([P, 2, D], FP16)

        # ---- persistent activation tiles ----
        kwt = [projp.tile([P, S], FP16, tag=f"kwt{p_}", name=f"kwt{p_}") for p_ in range(2)]
        qwt = [projp.tile([P, S], FP16, tag=f"qwt{p_}", name=f"qwt{p_}") for p_ in range(2)]
        vw = projp.tile([P, NTT, NV], BF16, tag="vw")
        ctx_t = [ctxp.tile([P, S], FP16, tag=f"ctx{p_}", name=f"ctx{p_}") for p_ in range(2)]

        def load_chunk(name, ci):
            xt = xtp.tile([P, NDC, SBLK], FP16, tag="xt", name="xt", bufs=5)
            nc.sync.dma_start(xt[:], xt_view[name][:, :, ci * SBLK:(ci + 1) * SBLK])
            return xt

        def proj_qk_piece(xt, w_sb, dst, bias_off, ci, pair, dc_range, pq_holder):
            """Part of one head-pair x one 512-s-chunk projection; the final
            piece adds the rank-1 bias and DVE-evicts to fp16 SBUF."""
            if dc_range[0] == 0:
                pq_holder[pair] = psum.tile([P, 2, SBLK], FP32, tag="sc",
                                            name="pq", bufs=3)
            pq = pq_holder[pair]
            for dc in dc_range:
                nc.tensor.matmul(pq[:, 0, :], lhsT=w_sb[:, dc, pair * P:(pair + 1) * P],
                                 rhs=xt[:, dc, :], start=(dc == 0), stop=False)
            if dc_range[-1] == NDC - 1:
                nc.tensor.matmul(
                    pq[:, 0, :],
                    lhsT=brow[:, bias_off + pair * P:bias_off + (pair + 1) * P],
                    rhs=brow[:, ONES_OFF:ONES_OFF + SBLK],
                    start=False, stop=True)
                nc.vector.tensor_copy(dst[pair][:, ci * SBLK:(ci + 1) * SBLK],
                                      pq[:, 0, :])

        def proj_qk(xt, w_sb, dst, bias_off, ci, pair):
            h = [None, None]
            proj_qk_piece(xt, w_sb, dst, bias_off, ci, pair, range(NDC), h)

        def proj_v_piece(xt, ci, c, dc_range, pv_holder):
            """Part of one t-tile of the natural-layout V projection."""
            tt = ci * (SBLK // P) + c
            if dc_range[0] == 0:
                pv_holder[0] = psum.tile([P, 2, SBLK], FP32, tag="sc",
                                         name="pv", bufs=3)
            pv = pv_holder[0]
            for dc in dc_range:
                nc.tensor.matmul(pv[:, 0, 0:NV], lhsT=xt[:, dc, c * P:(c + 1) * P],
                                 rhs=wv_sb[:, dc, :], start=(dc == 0), stop=False)
            if dc_range[-1] == NDC - 1:
                nc.tensor.matmul(pv[:, 0, 0:NV], lhsT=brow[:, ONES_OFF:ONES_OFF + P],
                                 rhs=brow[:, BVE_OFF:BVE_OFF + NV],
                                 start=False, stop=True)
                nc.vector.tensor_copy(vw[:, tt, :], pv[:, 0, 0:NV])

        def proj_v_tt(xt, ci, c):
            h = [None]
            proj_v_piece(xt, ci, c, range(NDC), h)

        def attn_block(pair, b, fillers):
            """Per-t-tile pipeline: scores(k) -> exp(k) -> ctx(k-2).
            One 2-bank scores PSUM per step (hp0 | hp1), 3-deep rotation;
            exp is a single FD=1024 ACT instruction. `fillers` is a list of
            (slot, fn); fn is emitted when the step index reaches slot."""
            ct = [psum.tile([DV + 1, SBLK], FP32, tag=f"ct{hp}", name=f"ct{hp}")
                  for hp in range(2)]
            exs = {}
            for k in range(NTT + 2):
                if k < NTT:
                    sc = psum.tile([P, 2, SBLK], FP32, tag="sc", name="sc", bufs=3)
                    for hp in range(2):
                        lo, hi = hp * DK, (hp + 1) * DK
                        nc.tensor.matmul(
                            sc[:, hp, :],
                            lhsT=kwt[pair][lo:hi, k * P:(k + 1) * P],
                            rhs=qwt[pair][lo:hi, b * SBLK:(b + 1) * SBLK],
                            start=True, stop=True)
                    ex = expp.tile([P, 2, SBLK], BF16, tag="ex", name="ex", bufs=3)
                    nc.scalar.activation(ex[:], sc[:],
                                         mybir.ActivationFunctionType.Exp)
                    exs[k] = ex
                while fillers and fillers[0][0] <= k:
                    fillers.pop(0)[1]()
                kc = k - 2
                if kc >= 0:
                    ex = exs.pop(kc)
                    for hp in range(2):
                        hh = 2 * pair + hp
                        nc.tensor.matmul(
                            ct[hp][:], lhsT=vw[:, kc, hh * (DV + 1):(hh + 1) * (DV + 1)],
                            rhs=ex[:, hp, :],
                            start=(kc == 0), stop=(kc == NTT - 1))
            return ct

        def attn_normalize(pair, b, ct, hp):
            # ctx = ct[0:64] * (1 / ct[64]) row-broadcast; fp16 out
            den = smallp.tile([1, SBLK], FP32R, tag="den")
            nc.vector.tensor_copy(den[:], ct[hp][DV:DV + 1, :])
            rb = psum.tile([P, 2, SBLK], FP32, tag="sc", name="rb", bufs=3)
            nc.tensor.matmul(rb[0:DV, 0, :], lhsT=onesdv[:],
                             rhs=den[:], start=True, stop=True)
            rcp = smallp.tile([DV, SBLK], FP32, tag="rcp")
            nc.vector.reciprocal_approx_fast(rcp[:], rb[0:DV, 0, :])
            nc.vector.tensor_mul(
                ctx_t[pair][hp * DV:(hp + 1) * DV, b * SBLK:(b + 1) * SBLK],
                ct[hp][0:DV, :], rcp[:])

        def out_proj_nh(b, st, nh):
            off = b * SBLK + st * P
            po = psum.tile([P, 2, SBLK], FP32, tag="sc", name="po", bufs=3)
            for jc in range(2):
                nc.tensor.matmul(po[:, 0, :],
                                 lhsT=ctx_t[jc][:, off:off + P],
                                 rhs=wo_sb[:, jc, nh * SBLK:(nh + 1) * SBLK],
                                 start=(jc == 0), stop=(jc == 1))
            ob = outp.tile([P, SBLK], FP16, tag="ob")
            nc.vector.tensor_copy(ob[:], po[:, 0, :])
            nc.sync.dma_start(out_d[off:off + P, nh * SBLK:(nh + 1) * SBLK], ob[:])

        # ---- emission schedule ----
        # Minimal prologue: K/Q chunk 0 land first and block 0 pair 0 starts
        # immediately; V chunk 0 feeds the (2-step-trailing) ctx matmuls.
        # Everything else — K/V chunks 1-3, Q chunks 1-3, out-proj,
        # normalize — drains into the per-step slack of the blocks as small
        # (<=4-matmul) filler units, slotted so each unit is emitted strictly
        # before its consumer but late enough that its DMA has landed (a
        # piece waiting on DMA at the PE queue head stalls everything).
        kt0 = load_chunk("kt", 0)
        qt0 = load_chunk("qt", 0)
        vt0 = load_chunk("vt", 0)
        proj_qk(kt0, wk_sb, kwt, BK_OFF, 0, 0)
        proj_qk(kt0, wk_sb, kwt, BK_OFF, 0, 1)
        proj_qk(qt0, wq_sb, qwt, BQ_OFF, 0, 0)
        proj_qk(qt0, wq_sb, qwt, BQ_OFF, 0, 1)
        kts = {1: load_chunk("kt", 1)}
        vts = {0: vt0, 1: load_chunk("vt", 1)}
        kts[2] = load_chunk("kt", 2)
        vts[2] = load_chunk("vt", 2)
        nc.sync.dma_start(wo_sb[:], d["wo"].rearrange("(jc p) n -> p jc n", p=P))
        qts = {0: qt0}

        def k_fillers(ci, pair, s0):
            """3 pieces: dc 0-2, 3-5, 6-7+bias+evict."""
            holder = [None, None]
            return [(s0 + j, lambda r=tuple(rr), h=holder, c=ci, p=pair:
                     proj_qk_piece(kts[c], wk_sb, kwt, BK_OFF, c, p, r, h))
                    for j, rr in enumerate(([0, 1, 2], [3, 4, 5], [6, 7]))]

        def q_fillers(ci, s0):
            out = []
            slot = s0
            for pair in range(2):
                holder = [None, None]
                for rr in ([0, 1, 2], [3, 4, 5], [6, 7]):
                    out.append((slot, lambda p=pair, r=tuple(rr), h=holder, c=ci:
                                proj_qk_piece(qts[c], wq_sb, qwt, BQ_OFF, c, p, r, h)))
                    slot += 1
            return out

        def v_fillers(ci, s0):
            """2 pieces per t-tile at slots (s0+c, s0+c+1): piece 2 lands one
            step before ctx(tt) consumes the tile (ctx trails by 2)."""
            out = []
            for c in range(4):
                holder = [None]
                for j, rr in enumerate(([0, 1, 2, 3], [4, 5, 6, 7])):
                    out.append((s0 + c + j,
                                lambda cc=c, r=tuple(rr), h=holder, ci_=ci:
                                proj_v_piece(vts[ci_], ci_, cc, r, h)))
            return out

        # b0p0: stream in K chunks 1-3 (pair 0) and all V chunks in slack.
        fill = sorted(
            v_fillers(0, 0)
            + k_fillers(1, 0, 1)
            + k_fillers(1, 1, 2)  # pair-1 c1 must fully consume kt1 before
                                  # vt3's DMA (slot 5) recycles its buffer
            + [(5, lambda: kts.__setitem__(3, load_chunk("kt", 3))),
               (6, lambda: vts.__setitem__(3, load_chunk("vt", 3)))]
            + k_fillers(2, 0, 4)
            + v_fillers(1, 4)
            + k_fillers(3, 0, 8)
            + v_fillers(2, 8)
            + v_fillers(3, 12),
            key=lambda x: x[0])
        ct = attn_block(0, 0, fill)
        prev = (0, 0, ct)

        # remaining 7 pair-blocks
        for b in range(NBLK):
            for pair in range(2):
                if b == 0 and pair == 0:
                    continue
                fill = []
                pp, pb, pct = prev
                fill.append((0, lambda p=pp, bb=pb, c=pct: attn_normalize(p, bb, c, 0)))
                fill.append((1, lambda p=pp, bb=pb, c=pct: attn_normalize(p, bb, c, 1)))
                if b == 0 and pair == 1:
                    # pair-1 K projections (chunk ci first needed by scores
                    # at step 4*ci, so pieces must sit at slots <= 4*ci-1)
                    fill += k_fillers(2, 1, 4)
                    fill += k_fillers(3, 1, 7)
                if pair == 1 and b < NBLK - 1:
                    # next block's Q chunk: DMA now, project late in block
                    qts[b + 1] = load_chunk("qt", b + 1)
                    fill += q_fillers(b + 1, 10)
                if pair == 0 and b > 0:
                    for u in range(8):
                        st, nh = u // 2, u % 2
                        fill.append((2 + u, lambda bb=b - 1, s=st, n=nh:
                                     out_proj_nh(bb, s, n)))
                fill.sort(key=lambda x: x[0])
                ct = attn_block(pair, b, fill)
                prev = (pair, b, ct)
        attn_normalize(1, NBLK - 1, ct, 0)
        attn_normalize(1, NBLK - 1, ct, 1)
        for u in range(8):
            out_proj_nh(NBLK - 1, u // 2, u % 2)

    nc.compile()
    return nc


_NC_CACHE = None


def _get_nc():
    global _NC_CACHE
    if _NC_CACHE is None:
        _NC_CACHE = _build_nc()
    return _NC_CACHE


def kernel(Q, K, V, Wq, bq, Wk, bk, Wv, bv, Wo, bo, _trace=False, _trace_kwargs=None):
    nc = _get_nc()
    qt_h = [np.ascontiguousarray(np.asarray(Q[b]).T).astype(np.float16) for b in range(B)]
    kt_h = [np.ascontiguousarray(np.asarray(K[b]).T).astype(np.float16) for b in range(B)]
    vt_h = [np.ascontiguousarray(np.asarray(V[b]).T).astype(np.float16) for b in range(B)]
    onesdv = np.ones((1, DV), dtype=np.float32)

    in_maps = []
    for c in range(N_CORES):
        b, g = c % B, c // B
        hs = list(range(g * HPC, (g + 1) * HPC))
        wq_p = np.concatenate([Wq[h] for h in hs], axis=1)
        wk_p = np.concatenate([Wk[h] for h in hs], axis=1)
        wv_e = np.zeros((D, NV), dtype=np.float32)
        bv_e = np.zeros(NV, dtype=np.float32)
        for i, h in enumerate(hs):
            wv_e[:, i * (DV + 1):i * (DV + 1) + DV] = Wv[h] * SCALE
            bv_e[i * (DV + 1):i * (DV + 1) + DV] = bv[h] * SCALE
            bv_e[i * (DV + 1) + DV] = 1.0
        brow = np.zeros((1, BROW_W), dtype=np.float32)
        brow[0, ONES_OFF:ONES_OFF + SBLK] = 1.0
        brow[0, BQ_OFF:BQ_OFF + 2 * P] = np.concatenate([bq[h] for h in hs])
        brow[0, BK_OFF:BK_OFF + 2 * P] = np.concatenate([bk[h] for h in hs])
        brow[0, BVE_OFF:BVE_OFF + NV] = bv_e
        in_maps.append({
            "qt": qt_h[b], "kt": kt_h[b], "vt": vt_h[b],
            "wq": np.ascontiguousarray(wq_p).astype(np.float16),
            "wk": np.ascontiguousarray(wk_p).astype(np.float16),
            "wv": np.ascontiguousarray(wv_e).astype(np.float16),
            "wo": np.ascontiguousarray(Wo[g * HPC * DV:(g + 1) * HPC * DV]).astype(np.float16),
            "brow": brow.astype(np.float16),
            "onesdv": onesdv,
        })

    kw = {}
    if _trace:
        kw = dict(trace=True, **(_trace_kwargs or {}))
    res = run_bass_kernel_spmd(nc, in_maps, core_ids=list(range(N_CORES)), **kw)

    out = np.zeros((B, S, D), dtype=np.float32)
    for c in range(N_CORES):
        out[c % B] += np.asarray(res.results[c]["out"], dtype=np.float32)
    out += bo[None, None, :]
    if _trace:
        return out, res
    return out


# revision 25
# speedup vs baseline: 1.5865x; 1.0085x over previous
"""Multi-head attention (B=2, S=2048, D=1024, H=16, dk=dv=64) on 8 TRN2 cores.

Sharding: core c -> batch b = c % 2, head-group g = c // 2 (heads 4g..4g+3).
Each core computes its 4 heads' attention for one batch plus the partial
output projection; the host sums the 4 partials per batch and adds bo.

v2 design (vs the transpose-heavy v1): the whole input path is fp16
(halves HBM traffic; fp16's 11-bit mantissa keeps scores to ~1e-3), the
V projection is computed directly in natural [t, dv] layout (lhsT = the
V chunk itself), and every projection bias is a rank-1 K=1 matmul into
the accumulating PSUM so all PSUM->SBUF evictions are pure DVE copies.
The ACT engine then does nothing but the softmax exp, which is its hard
floor: (1024+352)/1.2GHz per [128,1024] tile, ~147us over the 128 tiles.
The attention pipeline runs one t-tile per step (scores -> exp -> ctx
trailing 2 steps) with a 3-deep scores-PSUM rotation so the PE can run
ahead of ACT and never bubbles long enough to re-throttle the HAM clock
gate. K/V/Q chunks stream in while block 0 is already computing; Q-proj,
out-proj and normalize work drains into later blocks' per-step slack.
"""
import os
import sys

sys.path.insert(0, "/opt/trn_rl_repo")
os.environ.setdefault("JAX_PLATFORMS", "axon,cpu")

from contextlib import ExitStack

import numpy as np

import concourse.bacc as bacc
import concourse.tile as tile
from concourse import mybir
from concourse.bass_utils import run_bass_kernel_spmd

FP16 = mybir.dt.float16
BF16 = mybir.dt.bfloat16
FP32 = mybir.dt.float32
FP32R = mybir.dt.float32r

B, S, D = 2, 2048, 1024
H, DK, DV = 16, 64, 64
N_CORES = 8
HPC = H // (N_CORES // B)  # heads per core = 4
P = 128
SBLK = 512                # s-block (free dim of scores matmuls)
NBLK = S // SBLK          # 4
NTT = S // P              # 16 t-tiles
NDC = D // P              # 8 contraction chunks
NV = HPC * (DV + 1)       # 260 (64 V cols + 1 denominator-ones col per head)
SCALE = 1.0 / (DK * 2.0)  # folded into Wv/bv
# brow packing offsets (one [1, 1284] fp16 row of constants)
ONES_OFF, BQ_OFF, BK_OFF, BVE_OFF = 0, 512, 768, 1024
BROW_W = 1284


def _build_nc():
    nc = bacc.Bacc("TRN2", target_bir_lowering=False, debug=False,
                   num_devices=N_CORES)
    d = {}
    for name, shape, dt in [
        ("qt", [D, S], FP16), ("kt", [D, S], FP16), ("vt", [D, S], FP16),
        ("wq", [D, 2 * P], FP16), ("wk", [D, 2 * P], FP16),
        ("wv", [D, NV], FP16), ("wo", [HPC * DV, D], FP16),
        ("brow", [1, BROW_W], FP16), ("onesdv", [1, DV], FP32),
    ]:
        d[name] = nc.dram_tensor(name, shape, dt, kind="ExternalInput").ap()
    out_d = nc.dram_tensor("out", [S, D], FP16, kind="ExternalOutput").ap()
    xt_view = {
        n: d[n].rearrange("(dc p) s -> p dc s", p=P)
        for n in ("qt", "kt", "vt")
    }

    with tile.TileContext(nc) as tc, ExitStack() as ctx:
        const = ctx.enter_context(tc.tile_pool(name="const", bufs=1))
        wpool = ctx.enter_context(tc.tile_pool(name="wpool", bufs=1))
        xtp = ctx.enter_context(tc.tile_pool(name="xtp", bufs=1))
        projp = ctx.enter_context(tc.tile_pool(name="projp", bufs=1))
        expp = ctx.enter_context(tc.tile_pool(name="expp", bufs=1))
        ctxp = ctx.enter_context(tc.tile_pool(name="ctxp", bufs=1))
        outp = ctx.enter_context(tc.tile_pool(name="outp", bufs=2))
        smallp = ctx.enter_context(tc.tile_pool(name="smallp", bufs=2))
        psum = ctx.enter_context(tc.tile_pool(name="psum", bufs=1, space="PSUM"))

        # ---- ACT table warm-up: a 2-elem exp triggers ACT_TABLE_LOAD
        # while the first DMAs are still in flight.
        dummy = smallp.tile([1, 2], FP32, tag="dmy")
        dummy2 = smallp.tile([1, 2], FP32, tag="dmy2")
        nc.vector.memset(dummy[:], 0.0)
        nc.scalar.activation(dummy2[:], dummy[:],
                             mybir.ActivationFunctionType.Exp)

        # ---- constants / weights (sync queue: K/Q path; gpsimd queue: V) ----
        brow = const.tile([1, BROW_W], FP16)
        nc.sync.dma_start(brow[:], d["brow"])
        wk_sb = wpool.tile([P, NDC, 2 * P], FP16)
        nc.sync.dma_start(wk_sb[:], d["wk"].rearrange("(dc p) m -> p dc m", p=P))
        onesdv = const.tile([1, DV], FP32R)
        nc.gpsimd.dma_start(onesdv[:], d["onesdv"].bitcast(FP32R))
        wv_sb = wpool.tile([P, NDC, NV], FP16)
        nc.gpsimd.dma_start(wv_sb[:], d["wv"].rearrange("(dc p) m -> p dc m", p=P))
        wq_sb = wpool.tile([P, NDC, 2 * P], FP16)
        wo_sb = wpool.tile([P, 2, D], FP16)

        # ---- persistent activation tiles ----
        kwt = [projp.tile([P, S], FP16, tag=f"kwt{p_}", name=f"kwt{p_}") for p_ in range(2)]
        qwt = [projp.tile([P, S], FP16, tag=f"qwt{p_}", name=f"qwt{p_}") for p_ in range(2)]
        vw = projp.tile([P, NTT, NV], BF16, tag="vw")
        ctx_t = [ctxp.tile([P, S], FP16, tag=f"ctx{p_}", name=f"ctx{p_}") for p_ in range(2)]

        def load_chunk(name, ci, eng=None):
            # kt/qt ride the sync DMA queue, vt the gpsimd queue: two
            # parallel streams halve the DMA-gated prologue.
            xt = xtp.tile([P, NDC, SBLK], FP16, tag="xt", name="xt", bufs=5)
            (eng or nc.sync).dma_start(
                xt[:], xt_view[name][:, :, ci * SBLK:(ci + 1) * SBLK])
            return xt

        def proj_qk_piece(xt, w_sb, dst, bias_off, ci, pair, dc_range, pq_holder):
            """Part of one head-pair x one 512-s-chunk projection; the final
            piece adds the rank-1 bias and DVE-evicts to fp16 SBUF."""
            if dc_range[0] == 0:
                pq_holder[pair] = psum.tile([P, 2, SBLK], FP32, tag="sc",
                                            name="pq", bufs=3)
            pq = pq_holder[pair]
            for dc in dc_range:
                nc.tensor.matmul(pq[:, 0, :], lhsT=w_sb[:, dc, pair * P:(pair + 1) * P],
                                 rhs=xt[:, dc, :], start=(dc == 0), stop=False)
            if dc_range[-1] == NDC - 1:
                nc.tensor.matmul(
                    pq[:, 0, :],
                    lhsT=brow[:, bias_off + pair * P:bias_off + (pair + 1) * P],
                    rhs=brow[:, ONES_OFF:ONES_OFF + SBLK],
                    start=False, stop=True)
                nc.vector.tensor_copy(dst[pair][:, ci * SBLK:(ci + 1) * SBLK],
                                      pq[:, 0, :])

        def proj_qk(xt, w_sb, dst, bias_off, ci, pair):
            h = [None, None]
            proj_qk_piece(xt, w_sb, dst, bias_off, ci, pair, range(NDC), h)

        def proj_v_piece(xt, ci, c, dc_range, pv_holder):
            """Part of one t-tile of the natural-layout V projection."""
            tt = ci * (SBLK // P) + c
            if dc_range[0] == 0:
                pv_holder[0] = psum.tile([P, 2, SBLK], FP32, tag="sc",
                                         name="pv", bufs=3)
            pv = pv_holder[0]
            for dc in dc_range:
                nc.tensor.matmul(pv[:, 0, 0:NV], lhsT=xt[:, dc, c * P:(c + 1) * P],
                                 rhs=wv_sb[:, dc, :], start=(dc == 0), stop=False)
            if dc_range[-1] == NDC - 1:
                nc.tensor.matmul(pv[:, 0, 0:NV], lhsT=brow[:, ONES_OFF:ONES_OFF + P],
                                 rhs=brow[:, BVE_OFF:BVE_OFF + NV],
                                 start=False, stop=True)
                nc.vector.tensor_copy(vw[:, tt, :], pv[:, 0, 0:NV])

        def proj_v_tt(xt, ci, c):
            h = [None]
            proj_v_piece(xt, ci, c, range(NDC), h)

        def attn_block(pair, b, fillers):
            """Per-t-tile pipeline: scores(k) -> exp(k) -> ctx(k-2).
            One 2-bank scores PSUM per step (hp0 | hp1), 3-deep rotation;
            exp is a single FD=1024 ACT instruction. `fillers` is a list of
            (slot, fn); fn is emitted when the step index reaches slot."""
            ct = [psum.tile([DV + 1, SBLK], FP32, tag=f"ct{hp}", name=f"ct{hp}")
                  for hp in range(2)]
            exs = {}
            for k in range(NTT + 2):
                if k < NTT:
                    sc = psum.tile([P, 2, SBLK], FP32, tag="sc", name="sc", bufs=3)
                    for hp in range(2):
                        lo, hi = hp * DK, (hp + 1) * DK
                        nc.tensor.matmul(
                            sc[:, hp, :],
                            lhsT=kwt[pair][lo:hi, k * P:(k + 1) * P],
                            rhs=qwt[pair][lo:hi, b * SBLK:(b + 1) * SBLK],
                            start=True, stop=True)
                    ex = expp.tile([P, 2, SBLK], BF16, tag="ex", name="ex", bufs=3)
                    nc.scalar.activation(ex[:], sc[:],
                                         mybir.ActivationFunctionType.Exp)
                    exs[k] = ex
                while fillers and fillers[0][0] <= k:
                    fillers.pop(0)[1]()
                kc = k - 2
                if kc >= 0:
                    ex = exs.pop(kc)
                    for hp in range(2):
                        hh = 2 * pair + hp
                        nc.tensor.matmul(
                            ct[hp][:], lhsT=vw[:, kc, hh * (DV + 1):(hh + 1) * (DV + 1)],
                            rhs=ex[:, hp, :],
                            start=(kc == 0), stop=(kc == NTT - 1))
            return ct

        def attn_normalize(pair, b, ct, hp):
            # ctx = ct[0:64] * (1 / ct[64]) row-broadcast; fp16 out
            den = smallp.tile([1, SBLK], FP32R, tag="den")
            nc.vector.tensor_copy(den[:], ct[hp][DV:DV + 1, :])
            rb = psum.tile([P, 2, SBLK], FP32, tag="sc", name="rb", bufs=3)
            nc.tensor.matmul(rb[0:DV, 0, :], lhsT=onesdv[:],
                             rhs=den[:], start=True, stop=True)
            rcp = smallp.tile([DV, SBLK], FP32, tag="rcp")
            nc.vector.reciprocal_approx_fast(rcp[:], rb[0:DV, 0, :])
            nc.vector.tensor_mul(
                ctx_t[pair][hp * DV:(hp + 1) * DV, b * SBLK:(b + 1) * SBLK],
                ct[hp][0:DV, :], rcp[:])

        ob_holder = [None]

        def out_proj_st(b, st):
            """One s-tile of the output projection: [128 s, 1024 D] via 4
            N=512 matmuls (fp16 moving operand caps at 512); the 4 s-tiles
            of a block stage into one SBUF tile DMA'd out as a single 1MB
            transfer on the gpsimd queue."""
            off = b * SBLK + st * P
            if st == 0:
                ob_holder[0] = outp.tile([P, 4, D], FP16, tag="ob", name="ob")
            po = psum.tile([P, 2, SBLK], FP32, tag="sc", name="po", bufs=3)
            for nh in range(2):
                for jc in range(2):
                    nc.tensor.matmul(po[:, nh, :],
                                     lhsT=ctx_t[jc][:, off:off + P],
                                     rhs=wo_sb[:, jc, nh * SBLK:(nh + 1) * SBLK],
                                     start=(jc == 0), stop=(jc == 1))
            nc.vector.tensor_copy(ob_holder[0][:, st, :],
                                  po[:].rearrange("p u q -> p (u q)"))
            if st == 3:
                nc.gpsimd.dma_start(
                    out_d[b * SBLK:(b + 1) * SBLK, :].rearrange(
                        "(st p) n -> p st n", p=P),
                    ob_holder[0][:])

        # ---- emission schedule ----
        # Minimal prologue: K/Q chunk 0 land first and block 0 pair 0 starts
        # immediately; V chunk 0 feeds the (2-step-trailing) ctx matmuls.
        # Everything else — K/V chunks 1-3, Q chunks 1-3, out-proj,
        # normalize — drains into the per-step slack of the blocks as small
        # (<=4-matmul) filler units, slotted so each unit is emitted strictly
        # before its consumer but late enough that its DMA has landed (a
        # piece waiting on DMA at the PE queue head stalls everything).
        kt0 = load_chunk("kt", 0)
        nc.sync.dma_start(wq_sb[:], d["wq"].rearrange("(dc p) m -> p dc m", p=P))
        qt0 = load_chunk("qt", 0)
        vt0 = load_chunk("vt", 0, nc.gpsimd)
        proj_qk(kt0, wk_sb, kwt, BK_OFF, 0, 0)
        proj_qk(kt0, wk_sb, kwt, BK_OFF, 0, 1)
        proj_qk(qt0, wq_sb, qwt, BQ_OFF, 0, 0)
        proj_qk(qt0, wq_sb, qwt, BQ_OFF, 0, 1)
        kts = {1: load_chunk("kt", 1)}
        vts = {0: vt0, 1: load_chunk("vt", 1, nc.gpsimd)}
        kts[2] = load_chunk("kt", 2)
        vts[2] = load_chunk("vt", 2, nc.gpsimd)
        nc.sync.dma_start(wo_sb[:], d["wo"].rearrange("(jc p) n -> p jc n", p=P))
        qts = {0: qt0}

        def k_fillers(ci, pair, s0):
            """3 pieces: dc 0-2, 3-5, 6-7+bias+evict."""
            holder = [None, None]
            return [(s0 + j, lambda r=tuple(rr), h=holder, c=ci, p=pair:
                     proj_qk_piece(kts[c], wk_sb, kwt, BK_OFF, c, p, r, h))
                    for j, rr in enumerate(([0, 1, 2], [3, 4, 5], [6, 7]))]

        def q_fillers(ci, s0):
            out = []
            slot = s0
            for pair in range(2):
                holder = [None, None]
                for rr in ([0, 1, 2], [3, 4, 5], [6, 7]):
                    out.append((slot, lambda p=pair, r=tuple(rr), h=holder, c=ci:
                                proj_qk_piece(qts[c], wq_sb, qwt, BQ_OFF, c, p, r, h)))
                    slot += 1
            return out

        def v_fillers(ci, s0):
            """2 pieces per t-tile at slots (s0+c, s0+c+1): piece 2 lands one
            step before ctx(tt) consumes the tile (ctx trails by 2)."""
            out = []
            for c in range(4):
                holder = [None]
                for j, rr in enumerate(([0, 1, 2, 3], [4, 5, 6, 7])):
                    out.append((s0 + c + j,
                                lambda cc=c, r=tuple(rr), h=holder, ci_=ci:
                                proj_v_piece(vts[ci_], ci_, cc, r, h)))
            return out

        # b0p0: stream in K chunks 1-3 (pair 0) and all V chunks in slack.
        fill = sorted(
            v_fillers(0, 0)
            + k_fillers(1, 0, 1)
            + k_fillers(1, 1, 2)  # pair-1 c1 must fully consume kt1 before
                                  # vt3's DMA (slot 5) recycles its buffer
            + [(5, lambda: kts.__setitem__(3, load_chunk("kt", 3))),
               (6, lambda: vts.__setitem__(3, load_chunk("vt", 3, nc.gpsimd)))]
            + k_fillers(2, 0, 4)
            + v_fillers(1, 4)
            + k_fillers(3, 0, 8)
            + v_fillers(2, 8)
            + v_fillers(3, 12),
            key=lambda x: x[0])
        ct = attn_block(0, 0, fill)
        prev = (0, 0, ct)

        # remaining 7 pair-blocks
        for b in range(NBLK):
            for pair in range(2):
                if b == 0 and pair == 0:
                    continue
                fill = []
                pp, pb, pct = prev
                fill.append((0, lambda p=pp, bb=pb, c=pct: attn_normalize(p, bb, c, 0)))
                fill.append((1, lambda p=pp, bb=pb, c=pct: attn_normalize(p, bb, c, 1)))
                if b == 0 and pair == 1:
                    # pair-1 K projections (chunk ci first needed by scores
                    # at step 4*ci, so pieces must sit at slots <= 4*ci-1)
                    fill += k_fillers(2, 1, 4)
                    fill += k_fillers(3, 1, 7)
                if pair == 1 and b < NBLK - 1:
                    # next block's Q chunk: DMA now, project late in block
                    qts[b + 1] = load_chunk("qt", b + 1)
                    fill += q_fillers(b + 1, 10)
                if pair == 0 and b > 0:
                    for st in range(4):
                        fill.append((2 + st * 3, lambda bb=b - 1, s=st:
                                     out_proj_st(bb, s)))
                fill.sort(key=lambda x: x[0])
                ct = attn_block(pair, b, fill)
                prev = (pair, b, ct)
        attn_normalize(1, NBLK - 1, ct, 0)
        attn_normalize(1, NBLK - 1, ct, 1)
        for st in range(4):
            out_proj_st(NBLK - 1, st)

    nc.compile()
    return nc


_NC_CACHE = None


def _get_nc():
    global _NC_CACHE
    if _NC_CACHE is None:
        _NC_CACHE = _build_nc()
    return _NC_CACHE


def kernel(Q, K, V, Wq, bq, Wk, bk, Wv, bv, Wo, bo, _trace=False, _trace_kwargs=None):
    nc = _get_nc()
    qt_h = [np.ascontiguousarray(np.asarray(Q[b]).T).astype(np.float16) for b in range(B)]
    kt_h = [np.ascontiguousarray(np.asarray(K[b]).T).astype(np.float16) for b in range(B)]
    vt_h = [np.ascontiguousarray(np.asarray(V[b]).T).astype(np.float16) for b in range(B)]
    onesdv = np.ones((1, DV), dtype=np.float32)

    in_maps = []
    for c in range(N_CORES):
        b, g = c % B, c // B
        hs = list(range(g * HPC, (g + 1) * HPC))
        wq_p = np.concatenate([Wq[h] for h in hs], axis=1)
        wk_p = np.concatenate([Wk[h] for h in hs], axis=1)
        wv_e = np.zeros((D, NV), dtype=np.float32)
        bv_e = np.zeros(NV, dtype=np.float32)
        for i, h in enumerate(hs):
            wv_e[:, i * (DV + 1):i * (DV + 1) + DV] = Wv[h] * SCALE
            bv_e[i * (DV + 1):i * (DV + 1) + DV] = bv[h] * SCALE
            bv_e[i * (DV + 1) + DV] = 1.0
        brow = np.zeros((1, BROW_W), dtype=np.float32)
        brow[0, ONES_OFF:ONES_OFF + SBLK] = 1.0
        brow[0, BQ_OFF:BQ_OFF + 2 * P] = np.concatenate([bq[h] for h in hs])
        brow[0, BK_OFF:BK_OFF + 2 * P] = np.concatenate([bk[h] for h in hs])
        brow[0, BVE_OFF:BVE_OFF + NV] = bv_e
        in_maps.append({
            "qt": qt_h[b], "kt": kt_h[b], "vt": vt_h[b],
            "wq": np.ascontiguousarray(wq_p).astype(np.float16),
            "wk": np.ascontiguousarray(wk_p).astype(np.float16),
            "wv": np.ascontiguousarray(wv_e).astype(np.float16),
            "wo": np.ascontiguousarray(Wo[g * HPC * DV:(g + 1) * HPC * DV]).astype(np.float16),
            "brow": brow.astype(np.float16),
            "onesdv": onesdv,
        })

    kw = {}
    if _trace:
        kw = dict(trace=True, **(_trace_kwargs or {}))
    res = run_bass_kernel_spmd(nc, in_maps, core_ids=list(range(N_CORES)), **kw)

    out = np.zeros((B, S, D), dtype=np.float32)
    for c in range(N_CORES):
        out[c % B] += np.asarray(res.results[c]["out"], dtype=np.float32)
    out += bo[None, None, :]
    if _trace:
        return out, res
    return out


# revision 40
# speedup vs baseline: 1.6252x; 1.0244x over previous
"""Multi-head attention (B=2, S=2048, D=1024, H=16, dk=dv=64) on 8 TRN2 cores.

Sharding: core c -> batch b = c % 2, head-group g = c // 2 (heads 4g..4g+3).
Each core computes its 4 heads' attention for one batch plus the partial
output projection; the host sums the 4 partials per batch and adds bo.

v2 design (vs the transpose-heavy v1): the whole input path is fp16
(halves HBM traffic; fp16's 11-bit mantissa keeps scores to ~1e-3), the
V projection is computed directly in natural [t, dv] layout (lhsT = the
V chunk itself), and every projection bias is a rank-1 K=1 matmul into
the accumulating PSUM so all PSUM->SBUF evictions are pure DVE copies.
The ACT engine then does nothing but the softmax exp, which is its hard
floor: (1024+352)/1.2GHz per [128,1024] tile, ~147us over the 128 tiles.
The attention pipeline runs one t-tile per step (scores -> exp -> ctx
trailing 2 steps) with a 3-deep scores-PSUM rotation so the PE can run
ahead of ACT and never bubbles long enough to re-throttle the HAM clock
gate. K/V/Q chunks stream in while block 0 is already computing; Q-proj,
out-proj and normalize work drains into later blocks' per-step slack.
"""
import os
import sys

sys.path.insert(0, "/opt/trn_rl_repo")
os.environ.setdefault("JAX_PLATFORMS", "axon,cpu")

from contextlib import ExitStack

import numpy as np

import concourse.bacc as bacc
import concourse.tile as tile
from concourse import mybir
from concourse.bass_utils import run_bass_kernel_spmd

FP16 = mybir.dt.float16
BF16 = mybir.dt.bfloat16
FP32 = mybir.dt.float32
FP32R = mybir.dt.float32r

B, S, D = 2, 2048, 1024
H, DK, DV = 16, 64, 64
N_CORES = 8
HPC = H // (N_CORES // B)  # heads per core = 4
P = 128
SBLK = 512                # s-block (free dim of scores matmuls)
NBLK = S // SBLK          # 4
NTT = S // P              # 16 t-tiles
NDC = D // P              # 8 contraction chunks
NV = HPC * (DV + 1)       # 260 (64 V cols + 1 denominator-ones col per head)
SCALE = 1.0 / (DK * 2.0)  # folded into Wv/bv
# brow packing offsets (one [1, 1284] fp16 row of constants)
ONES_OFF, BQ_OFF, BK_OFF, BVE_OFF = 0, 512, 768, 1024
BROW_W = 1284


def _build_nc():
    nc = bacc.Bacc("TRN2", target_bir_lowering=False, debug=False,
                   num_devices=N_CORES)
    # All bulk tensors are host-permuted so every DMA line is one long
    # contiguous row per partition (128 descriptors per transfer, not 1024
    # 1KB ones — descriptor generation was serializing the DMA queues).
    d = {}
    for name, shape, dt in [
        ("qt", [NBLK, P, NDC * SBLK], FP16), ("kt", [NBLK, P, NDC * SBLK], FP16),
        ("vt", [NBLK, P, NDC * SBLK], FP16),
        ("wq", [P, NDC * 2 * P], FP16), ("wk", [P, NDC * 2 * P], FP16),
        ("wv", [P, NDC * NV], FP16), ("wo", [P, 2 * D], FP16),
        ("brow", [1, BROW_W], FP16), ("onesdv", [1, DV], FP32),
    ]:
        d[name] = nc.dram_tensor(name, shape, dt, kind="ExternalInput").ap()
    out_d = nc.dram_tensor("out", [NBLK, P, 4 * D], FP16, kind="ExternalOutput").ap()

    with tile.TileContext(nc) as tc, ExitStack() as ctx:
        const = ctx.enter_context(tc.tile_pool(name="const", bufs=1))
        wpool = ctx.enter_context(tc.tile_pool(name="wpool", bufs=1))
        xtp = ctx.enter_context(tc.tile_pool(name="xtp", bufs=1))
        projp = ctx.enter_context(tc.tile_pool(name="projp", bufs=1))
        expp = ctx.enter_context(tc.tile_pool(name="expp", bufs=1))
        ctxp = ctx.enter_context(tc.tile_pool(name="ctxp", bufs=1))
        outp = ctx.enter_context(tc.tile_pool(name="outp", bufs=2))
        smallp = ctx.enter_context(tc.tile_pool(name="smallp", bufs=2))
        psum = ctx.enter_context(tc.tile_pool(name="psum", bufs=1, space="PSUM"))

        # ---- ACT table warm-up: a 2-elem exp triggers ACT_TABLE_LOAD
        # while the first DMAs are still in flight.
        dummy = smallp.tile([1, 2], FP32, tag="dmy")
        dummy2 = smallp.tile([1, 2], FP32, tag="dmy2")
        nc.vector.memset(dummy[:], 0.0)
        nc.scalar.activation(dummy2[:], dummy[:],
                             mybir.ActivationFunctionType.Exp)

        # ---- constants / weights (sync queue: K/Q path; gpsimd queue: V) ----
        brow = const.tile([1, BROW_W], FP16)
        nc.sync.dma_start(brow[:], d["brow"])
        wk_sb = wpool.tile([P, NDC, 2 * P], FP16)
        nc.sync.dma_start(wk_sb[:].rearrange("p a b -> p (a b)"), d["wk"])
        onesdv = const.tile([1, DV], FP32R)
        nc.gpsimd.dma_start(onesdv[:], d["onesdv"].bitcast(FP32R))
        wv_sb = wpool.tile([P, NDC, NV], FP16)
        nc.gpsimd.dma_start(wv_sb[:].rearrange("p a b -> p (a b)"), d["wv"])
        wq_sb = wpool.tile([P, NDC, 2 * P], FP16)
        wo_sb = wpool.tile([P, 2, D], FP16)

        # ---- persistent activation tiles ----
        kwt = [projp.tile([P, S], FP16, tag=f"kwt{p_}", name=f"kwt{p_}") for p_ in range(2)]
        qwt = [projp.tile([P, S], FP16, tag=f"qwt{p_}", name=f"qwt{p_}") for p_ in range(2)]
        vw = projp.tile([P, NTT, NV], BF16, tag="vw")
        ctx_t = [ctxp.tile([P, S], FP16, tag=f"ctx{p_}", name=f"ctx{p_}") for p_ in range(2)]

        def load_chunk(name, ci, eng=None):
            # kt/qt ride the sync DMA queue, vt the gpsimd queue: two
            # parallel streams halve the DMA-gated prologue.
            xt = xtp.tile([P, NDC, SBLK], FP16, tag="xt", name="xt", bufs=5)
            (eng or nc.sync).dma_start(
                xt[:].rearrange("p a b -> p (a b)"), d[name][ci])
            return xt

        def proj_qk_piece(xt, w_sb, dst, bias_off, ci, pair, dc_range, pq_holder):
            """Part of one head-pair x one 512-s-chunk projection; the final
            piece adds the rank-1 bias and DVE-evicts to fp16 SBUF."""
            if dc_range[0] == 0:
                pq_holder[pair] = psum.tile([P, 2, SBLK], FP32, tag="sc",
                                            name="pq", bufs=3)
            pq = pq_holder[pair]
            for dc in dc_range:
                nc.tensor.matmul(pq[:, 0, :], lhsT=w_sb[:, dc, pair * P:(pair + 1) * P],
                                 rhs=xt[:, dc, :], start=(dc == 0), stop=False)
            if dc_range[-1] == NDC - 1:
                nc.tensor.matmul(
                    pq[:, 0, :],
                    lhsT=brow[:, bias_off + pair * P:bias_off + (pair + 1) * P],
                    rhs=brow[:, ONES_OFF:ONES_OFF + SBLK],
                    start=False, stop=True)
                nc.vector.tensor_copy(dst[pair][:, ci * SBLK:(ci + 1) * SBLK],
                                      pq[:, 0, :])

        def proj_qk(xt, w_sb, dst, bias_off, ci, pair):
            h = [None, None]
            proj_qk_piece(xt, w_sb, dst, bias_off, ci, pair, range(NDC), h)

        def proj_v_piece(xt, ci, c, dc_range, pv_holder):
            """Part of one t-tile of the natural-layout V projection."""
            tt = ci * (SBLK // P) + c
            if dc_range[0] == 0:
                pv_holder[0] = psum.tile([P, 2, SBLK], FP32, tag="sc",
                                         name="pv", bufs=3)
            pv = pv_holder[0]
            for dc in dc_range:
                nc.tensor.matmul(pv[:, 0, 0:NV], lhsT=xt[:, dc, c * P:(c + 1) * P],
                                 rhs=wv_sb[:, dc, :], start=(dc == 0), stop=False)
            if dc_range[-1] == NDC - 1:
                nc.tensor.matmul(pv[:, 0, 0:NV], lhsT=brow[:, ONES_OFF:ONES_OFF + P],
                                 rhs=brow[:, BVE_OFF:BVE_OFF + NV],
                                 start=False, stop=True)
                nc.vector.tensor_copy(vw[:, tt, :], pv[:, 0, 0:NV])

        def proj_v_tt(xt, ci, c):
            h = [None]
            proj_v_piece(xt, ci, c, range(NDC), h)

        def attn_block(pair, b, fillers):
            """Per-t-tile pipeline: scores(k) -> exp(k) -> ctx(k-2).
            One 2-bank scores PSUM per step (hp0 | hp1), 3-deep rotation;
            exp is a single FD=1024 ACT instruction. `fillers` is a list of
            (slot, fn); fn is emitted when the step index reaches slot."""
            ct = [psum.tile([DV + 1, SBLK], FP32, tag=f"ct{hp}", name=f"ct{hp}")
                  for hp in range(2)]
            exs = {}
            for k in range(NTT + 2):
                if k < NTT:
                    sc = psum.tile([P, 2, SBLK], FP32, tag="sc", name="sc", bufs=3)
                    for hp in range(2):
                        lo, hi = hp * DK, (hp + 1) * DK
                        nc.tensor.matmul(
                            sc[:, hp, :],
                            lhsT=kwt[pair][lo:hi, k * P:(k + 1) * P],
                            rhs=qwt[pair][lo:hi, b * SBLK:(b + 1) * SBLK],
                            start=True, stop=True)
                    ex = expp.tile([P, 2, SBLK], BF16, tag="ex", name="ex", bufs=3)
                    nc.scalar.activation(ex[:], sc[:],
                                         mybir.ActivationFunctionType.Exp)
                    exs[k] = ex
                while fillers and fillers[0][0] <= k:
                    fillers.pop(0)[1]()
                kc = k - 2
                if kc >= 0:
                    ex = exs.pop(kc)
                    for hp in range(2):
                        hh = 2 * pair + hp
                        nc.tensor.matmul(
                            ct[hp][:], lhsT=vw[:, kc, hh * (DV + 1):(hh + 1) * (DV + 1)],
                            rhs=ex[:, hp, :],
                            start=(kc == 0), stop=(kc == NTT - 1))
            return ct

        def attn_normalize(pair, b, ct, hp):
            # ctx = ct[0:64] * (1 / ct[64]) row-broadcast; fp16 out
            den = smallp.tile([1, SBLK], FP32R, tag="den")
            nc.vector.tensor_copy(den[:], ct[hp][DV:DV + 1, :])
            rb = psum.tile([P, 2, SBLK], FP32, tag="sc", name="rb", bufs=3)
            nc.tensor.matmul(rb[0:DV, 0, :], lhsT=onesdv[:],
                             rhs=den[:], start=True, stop=True)
            rcp = smallp.tile([DV, SBLK], FP32, tag="rcp")
            nc.vector.reciprocal_approx_fast(rcp[:], rb[0:DV, 0, :])
            nc.vector.tensor_mul(
                ctx_t[pair][hp * DV:(hp + 1) * DV, b * SBLK:(b + 1) * SBLK],
                ct[hp][0:DV, :], rcp[:])

        ob_holder = [None]

        def out_proj_st(b, st):
            """One s-tile of the output projection: [128 s, 1024 D] via 4
            N=512 matmuls (fp16 moving operand caps at 512); the 4 s-tiles
            of a block stage into one SBUF tile DMA'd out as a single 1MB
            transfer on the gpsimd queue."""
            off = b * SBLK + st * P
            if st == 0:
                ob_holder[0] = outp.tile([P, 4, D], FP16, tag="ob", name="ob")
            po = psum.tile([P, 2, SBLK], FP32, tag="sc", name="po", bufs=3)
            for nh in range(2):
                for jc in range(2):
                    nc.tensor.matmul(po[:, nh, :],
                                     lhsT=ctx_t[jc][:, off:off + P],
                                     rhs=wo_sb[:, jc, nh * SBLK:(nh + 1) * SBLK],
                                     start=(jc == 0), stop=(jc == 1))
            nc.vector.tensor_copy(ob_holder[0][:, st, :],
                                  po[:].rearrange("p u q -> p (u q)"))
            if st == 3:
                nc.gpsimd.dma_start(out_d[b],
                                    ob_holder[0][:].rearrange("p a b -> p (a b)"))

        # ---- emission schedule ----
        # Minimal prologue: K/Q chunk 0 land first and block 0 pair 0 starts
        # immediately; V chunk 0 feeds the (2-step-trailing) ctx matmuls.
        # Everything else — K/V chunks 1-3, Q chunks 1-3, out-proj,
        # normalize — drains into the per-step slack of the blocks as small
        # (<=4-matmul) filler units, slotted so each unit is emitted strictly
        # before its consumer but late enough that its DMA has landed (a
        # piece waiting on DMA at the PE queue head stalls everything).
        kt0 = load_chunk("kt", 0)
        nc.sync.dma_start(wq_sb[:].rearrange("p a b -> p (a b)"), d["wq"])
        qt0 = load_chunk("qt", 0)
        vt0 = load_chunk("vt", 0, nc.gpsimd)
        proj_qk(kt0, wk_sb, kwt, BK_OFF, 0, 0)
        proj_qk(kt0, wk_sb, kwt, BK_OFF, 0, 1)
        proj_qk(qt0, wq_sb, qwt, BQ_OFF, 0, 0)
        proj_qk(qt0, wq_sb, qwt, BQ_OFF, 0, 1)
        kts = {1: load_chunk("kt", 1)}
        vts = {0: vt0, 1: load_chunk("vt", 1, nc.gpsimd)}
        kts[2] = load_chunk("kt", 2)
        vts[2] = load_chunk("vt", 2, nc.gpsimd)
        nc.sync.dma_start(wo_sb[:].rearrange("p a b -> p (a b)"), d["wo"])
        qts = {0: qt0}

        def k_fillers(ci, pair, s0):
            """3 pieces: dc 0-2, 3-5, 6-7+bias+evict."""
            holder = [None, None]
            return [(s0 + j, lambda r=tuple(rr), h=holder, c=ci, p=pair:
                     proj_qk_piece(kts[c], wk_sb, kwt, BK_OFF, c, p, r, h))
                    for j, rr in enumerate(([0, 1, 2], [3, 4, 5], [6, 7]))]

        def q_fillers(ci, s0):
            out = []
            slot = s0
            for pair in range(2):
                holder = [None, None]
                for rr in ([0, 1, 2], [3, 4, 5], [6, 7]):
                    out.append((slot, lambda p=pair, r=tuple(rr), h=holder, c=ci:
                                proj_qk_piece(qts[c], wq_sb, qwt, BQ_OFF, c, p, r, h)))
                    slot += 1
            return out

        def v_fillers(ci, s0):
            """2 pieces per t-tile at slots (s0+c, s0+c+1): piece 2 lands one
            step before ctx(tt) consumes the tile (ctx trails by 2)."""
            out = []
            for c in range(4):
                holder = [None]
                for j, rr in enumerate(([0, 1, 2, 3], [4, 5, 6, 7])):
                    out.append((s0 + c + j,
                                lambda cc=c, r=tuple(rr), h=holder, ci_=ci:
                                proj_v_piece(vts[ci_], ci_, cc, r, h)))
            return out

        # b0p0: stream in K chunks 1-3 (pair 0) and all V chunks in slack.
        fill = sorted(
            v_fillers(0, 0)
            + k_fillers(1, 0, 1)
            + k_fillers(1, 1, 2)  # pair-1 c1 must fully consume kt1 before
                                  # vt3's DMA (slot 5) recycles its buffer
            + [(5, lambda: kts.__setitem__(3, load_chunk("kt", 3))),
               (6, lambda: vts.__setitem__(3, load_chunk("vt", 3, nc.gpsimd)))]
            + k_fillers(2, 0, 4)
            + v_fillers(1, 4)
            + k_fillers(3, 0, 8)
            + v_fillers(2, 8)
            + v_fillers(3, 12),
            key=lambda x: x[0])
        ct = attn_block(0, 0, fill)
        prev = (0, 0, ct)

        # remaining 7 pair-blocks
        for b in range(NBLK):
            for pair in range(2):
                if b == 0 and pair == 0:
                    continue
                fill = []
                pp, pb, pct = prev
                fill.append((0, lambda p=pp, bb=pb, c=pct: attn_normalize(p, bb, c, 0)))
                fill.append((1, lambda p=pp, bb=pb, c=pct: attn_normalize(p, bb, c, 1)))
                if b == 0 and pair == 1:
                    # pair-1 K projections (chunk ci first needed by scores
                    # at step 4*ci, so pieces must sit at slots <= 4*ci-1)
                    fill += k_fillers(2, 1, 4)
                    fill += k_fillers(3, 1, 7)
                if pair == 1 and b < NBLK - 1:
                    # next block's Q chunk: DMA now, project late in block
                    qts[b + 1] = load_chunk("qt", b + 1)
                    fill += q_fillers(b + 1, 10)
                if pair == 0 and b > 0:
                    for st in range(4):
                        fill.append((2 + st * 3, lambda bb=b - 1, s=st:
                                     out_proj_st(bb, s)))
                fill.sort(key=lambda x: x[0])
                ct = attn_block(pair, b, fill)
                prev = (pair, b, ct)
        attn_normalize(1, NBLK - 1, ct, 0)
        attn_normalize(1, NBLK - 1, ct, 1)
        for st in range(4):
            out_proj_st(NBLK - 1, st)

    nc.compile()
    return nc


_NC_CACHE = None


def _get_nc():
    global _NC_CACHE
    if _NC_CACHE is None:
        _NC_CACHE = _build_nc()
    return _NC_CACHE


def _chunked(xT):
    """[D, S] -> [NBLK, P, NDC*SBLK]: chunk tile (p, dc, s) = xT[dc*128+p,
    ci*512+s], laid out so each partition's chunk row is contiguous."""
    x = xT.reshape(NDC, P, NBLK, SBLK).transpose(2, 1, 0, 3)
    return np.ascontiguousarray(x.reshape(NBLK, P, NDC * SBLK)).astype(np.float16)


def _wpack(w, cols):
    """[D, cols] -> [P, NDC*cols] with (p, dc, m) = w[dc*128+p, m]."""
    x = w.reshape(NDC, P, cols).transpose(1, 0, 2)
    return np.ascontiguousarray(x.reshape(P, NDC * cols)).astype(np.float16)


def kernel(Q, K, V, Wq, bq, Wk, bk, Wv, bv, Wo, bo, _trace=False, _trace_kwargs=None):
    nc = _get_nc()
    qt_h = [_chunked(np.asarray(Q[b]).T) for b in range(B)]
    kt_h = [_chunked(np.asarray(K[b]).T) for b in range(B)]
    vt_h = [_chunked(np.asarray(V[b]).T) for b in range(B)]
    onesdv = np.ones((1, DV), dtype=np.float32)

    in_maps = []
    for c in range(N_CORES):
        b, g = c % B, c // B
        hs = list(range(g * HPC, (g + 1) * HPC))
        wq_p = np.concatenate([Wq[h] for h in hs], axis=1)
        wk_p = np.concatenate([Wk[h] for h in hs], axis=1)
        wv_e = np.zeros((D, NV), dtype=np.float32)
        bv_e = np.zeros(NV, dtype=np.float32)
        for i, h in enumerate(hs):
            wv_e[:, i * (DV + 1):i * (DV + 1) + DV] = Wv[h] * SCALE
            bv_e[i * (DV + 1):i * (DV + 1) + DV] = bv[h] * SCALE
            bv_e[i * (DV + 1) + DV] = 1.0
        brow = np.zeros((1, BROW_W), dtype=np.float32)
        brow[0, ONES_OFF:ONES_OFF + SBLK] = 1.0
        brow[0, BQ_OFF:BQ_OFF + 2 * P] = np.concatenate([bq[h] for h in hs])
        brow[0, BK_OFF:BK_OFF + 2 * P] = np.concatenate([bk[h] for h in hs])
        brow[0, BVE_OFF:BVE_OFF + NV] = bv_e
        wo_g = np.asarray(Wo[g * HPC * DV:(g + 1) * HPC * DV])  # [256, 1024]
        wo_p = wo_g.reshape(2, P, D).transpose(1, 0, 2).reshape(P, 2 * D)
        in_maps.append({
            "qt": qt_h[b], "kt": kt_h[b], "vt": vt_h[b],
            "wq": _wpack(wq_p, 2 * P),
            "wk": _wpack(wk_p, 2 * P),
            "wv": _wpack(wv_e, NV),
            "wo": np.ascontiguousarray(wo_p).astype(np.float16),
            "brow": brow.astype(np.float16),
            "onesdv": onesdv,
        })

    kw = {}
    if _trace:
        kw = dict(trace=True, **(_trace_kwargs or {}))
    res = run_bass_kernel_spmd(nc, in_maps, core_ids=list(range(N_CORES)), **kw)

    out = np.zeros((B, S, D), dtype=np.float32)
    for c in range(N_CORES):
        o = np.asarray(res.results[c]["out"], dtype=np.float32)
        # [NBLK, P, 4*D]: row s = b*512 + st*128 + p
        o = o.reshape(NBLK, P, 4, D).transpose(0, 2, 1, 3).reshape(S, D)
        out[c % B] += o
    out += bo[None, None, :]
    if _trace:
        return out, res
    return out
